# revision 3
# baseline (speedup 1.0000x reference)
"""AFT-Full attention kernel for 8 TRN2 NeuronCores.

Math: the reference's exp_pos_bias = exp(pos_bias - max(pos_bias, axis=0)) is
identically 1.0 (the max is over a singleton dim), so the two (b,Ti,Tj,Dh)
einsums collapse to per-(b,h) sums over j:
    num[b,h] = sum_j exp(K-max_b K)[b,j,h] * V[b,j,h]
    den[b,h] = sum_j exp(K-max_b K)[b,j,h]
    out = (sigmoid(Q) * num/den) @ Wo.T

Sharding: sequence-parallel over T (256 positions per core, all 4 batches),
weights replicated.  Per-core row index r = b*256 + t_local (b-major) so the
per-(b,h) sums over t are contiguous innermost reduces.  One AllReduce per
half of d_hidden combines the per-core partial num/den.

Schedule (v2): K/V projections run as hb-PAIRS with the contraction (mc)
loop OUTER so the first matmuls start as soon as the first 128-row chunk of
kt/wk lands (instead of waiting for the full 4MB).  num/den for hb 0-3
complete after pair 1 -> AllReduce half 0 doorbell rings ~35us earlier than
the v1 schedule; the collectives entry barrier ends when the LAST core rings
its first doorbell, so earlier doorbells move the whole collective chain
earlier.  The output projection runs in 4 phases so the PE never idles
between the two AllReduce completions:
  O1: rblk 0-3 x hc 0-3 (needs r half 0), spilled to SBUF (frees PSUM)
  O2: rblk 4-7 x hc 0-3, held open in PSUM across the AR1 wait
  O3: rblk 4-7 x hc 4-7 finish + store          (needs r half 1)
  O4: rblk 0-3 x hc 4-7 + DVE add of O1 spill + store
"""

import numpy as np
import ml_dtypes

import concourse.bass as bass
import concourse.mybir as mybir
import concourse.tile as tile
from concourse import bacc
from concourse.bass_utils import run_bass_kernel_spmd

B, T, DM, DH = 4, 2048, 1024, 1024
N_CORES = 8
TC = T // N_CORES          # 256 sequence positions per core
R = B * TC                 # 1024 rows per core, r = b*256 + t
P = 128
MC = DM // P               # 8 contraction chunks (d_model)
HB = DH // P               # 8 hidden blocks
NB = 512                   # matmul moving free dim
RB = R // NB               # 2 row blocks
MB = DM // NB              # 2 output-model blocks

F16 = mybir.dt.bfloat16
F8 = mybir.dt.float8e4
F32 = mybir.dt.float32
NPF16 = ml_dtypes.bfloat16
NPF8 = ml_dtypes.float8_e4m3   # TRN FP8_EXP4: max +-240, matches in range
# fp8 pre-scales for the Q path (descaled inside the sigmoid activation).
# q ~ N(0,1) * 16 -> +-88 max; Wq ~ N(0,0.02^2) * 512 -> +-56 max: no clip,
# negligible subnormals.  Sigmoid damps the fp8 quantization 4x; simulated
# end-to-end rel err 1.07e-2 vs the 2e-2 gate.
QSCALE = 16.0
WQSCALE = 512.0

_GRAPH = None


def _body(nc, tc):
    qT = nc.dram_tensor("qT", [DM, R], F8, kind="ExternalInput").ap()
    kT = nc.dram_tensor("kT", [DM, R], F16, kind="ExternalInput").ap()
    vT = nc.dram_tensor("vT", [DM, R], F16, kind="ExternalInput").ap()
    wqT = nc.dram_tensor("wqT", [DM, DH], F8, kind="ExternalInput").ap()
    wkT = nc.dram_tensor("wkT", [DM, DH], F16, kind="ExternalInput").ap()
    wvT = nc.dram_tensor("wvT", [DM, DH], F16, kind="ExternalInput").ap()
    woT = nc.dram_tensor("woT", [DH, DM], F16, kind="ExternalInput").ap()
    out = nc.dram_tensor("out", [R, DM], F32, kind="ExternalOutput").ap()

    Exp = mybir.ActivationFunctionType.Exp
    Sigmoid = mybir.ActivationFunctionType.Sigmoid
    Op = mybir.AluOpType
    DR = mybir.MatmulPerfMode.DoubleRow

    from contextlib import ExitStack
    with ExitStack() as ctx:
        acts = ctx.enter_context(tc.tile_pool(name="acts", bufs=1))
        work = ctx.enter_context(tc.tile_pool(name="work", bufs=2))
        sqp = ctx.enter_context(tc.tile_pool(name="sqp", bufs=1))
        obp = ctx.enter_context(tc.tile_pool(name="obp", bufs=1))
        # 2 + 2 [128,1024]f32 buffers = all 8 PSUM banks.  K pairs use the
        # two "k" slots, V pairs the two "v" slots; the O phases use all 4.
        psk = ctx.enter_context(tc.tile_pool(name="psk", bufs=2, space="PSUM"))
        psv = ctx.enter_context(tc.tile_pool(name="psv", bufs=2, space="PSUM"))
        dram = ctx.enter_context(tc.tile_pool(name="dram", bufs=1, space="DRAM"))

        # Warmup matmuls on zeros un-throttle the PE clock (HAM) while the
        # first input chunks stream in; a copy of their result is stored to
        # a DRAM scratch tile so they are not dead code.
        warm = acts.tile([P, 640], F16, name="warm")
        nc.gpsimd.memset(warm[:], 0.0)
        pwu = psk.tile([P, NB], F32, name="pwu", tag="k")
        pwu2 = psv.tile([P, NB], F32, name="pwu2", tag="v")
        for i in range(8):
            t = pwu if i % 2 == 0 else pwu2
            nc.tensor.matmul(t[:], warm[:, 0:P], warm[:, P:P + NB],
                             start=True, stop=True)
        wdump = acts.tile([1, 2], F32, name="wdump")
        nc.vector.tensor_copy(wdump[:, 0:1], pwu[0:1, 0:1])
        nc.vector.tensor_copy(wdump[:, 1:2], pwu2[0:1, 0:1])
        wscratch = dram.tile([1, 2], F32, name="wscratch")
        nc.sync.dma_start(wscratch[:], wdump[:])

        def declare(name, free, dt=F16):
            return acts.tile([P, MC, free], dt, name=name)

        def load_chunk(t, ap_dram, mc, eng):
            src = ap_dram.rearrange("(c p) f -> p c f", p=P)
            eng.dma_start(t[:, mc, :], src[:, mc, :])

        # K/V operands stream in per-mc; the mc-outer matmul emission below
        # consumes each chunk as it lands.  The two HWDGE queues (sync=SP,
        # scalar=ACT) each sustain ~200 GB/s, so the K stream rides sync
        # and the V stream rides scalar concurrently.
        kt = declare("kt", R)
        wk = declare("wk", DH)
        vt = declare("vt", R)
        wv = declare("wv", DH)
        for mc in range(MC):
            load_chunk(wk, wkT, mc, nc.sync)
            load_chunk(kt, kT, mc, nc.sync)
            load_chunk(wv, wvT, mc, nc.scalar)
            load_chunk(vt, vT, mc, nc.scalar)
        qt = declare("qt", R, F8)
        nc.sync.dma_start(qt[:], qT.rearrange("(c p) f -> p c f", p=P))
        wq = declare("wq", DH, F8)
        nc.sync.dma_start(wq[:], wqT.rearrange("(c p) f -> p c f", p=P))
        wo = declare("wo", DM)
        nc.scalar.dma_start(wo[:], woT.rearrange("(c p) f -> p c f", p=P))

        nd_all = acts.tile([P, HB, 8], F32, name="nd_all")

        def kpost(hb, pk):
            # max over batch: copy + 3 maxes (DVE may read PSUM only once/op)
            mk = work.tile([P, TC], F32, name="mk")
            nc.vector.tensor_copy(mk[:], pk[:, 0 * TC:1 * TC])
            for b in range(1, B):
                nc.vector.tensor_tensor(
                    mk[:], mk[:], pk[:, b * TC:(b + 1) * TC], op=Op.max)
            ek = work.tile([P, R], F32, name="ek")
            ek3 = ek.rearrange("p (b t) -> p b t", t=TC)
            nc.vector.tensor_tensor(
                ek3, pk.rearrange("p (b t) -> p b t", t=TC),
                mk[:, None, :].to_broadcast((P, B, TC)), op=Op.subtract)
            # exp on ACT with fused per-b den accumulation (contiguous slices)
            for b in range(B):
                bs = slice(b * TC, (b + 1) * TC)
                nc.scalar.activation(
                    ek[:, bs], ek[:, bs], Exp,
                    accum_out=nd_all[:, hb, b:b + 1])
            return ek

        # ---- K pair: projections for hb-pair (2p, 2p+1) with mc OUTER so
        # the matmuls chase the kt/wk DMA stream chunk by chunk. ----
        def kpair(p):
            pks = [psk.tile([P, R], F32, name="pk", tag="k") for _ in range(2)]
            for mc in range(MC):
                for i in range(2):
                    hs = slice((2 * p + i) * P, (2 * p + i + 1) * P)
                    for rb in range(RB):
                        nc.tensor.matmul(
                            pks[i][:, rb * NB:(rb + 1) * NB],
                            wk[:, mc, hs],
                            kt[:, mc, rb * NB:(rb + 1) * NB],
                            start=(mc == 0), stop=(mc == MC - 1),
                        )
            return [kpost(2 * p + i, pks[i]) for i in range(2)]

        def vpair(p, eks):
            pvs = [psv.tile([P, R], F32, name="pv", tag="v") for _ in range(2)]
            for mc in range(MC):
                for i in range(2):
                    hs = slice((2 * p + i) * P, (2 * p + i + 1) * P)
                    for rb in range(RB):
                        nc.tensor.matmul(
                            pvs[i][:, rb * NB:(rb + 1) * NB],
                            wv[:, mc, hs],
                            vt[:, mc, rb * NB:(rb + 1) * NB],
                            start=(mc == 0), stop=(mc == MC - 1),
                        )
            for i in range(2):
                hb = 2 * p + i
                ekv = work.tile([P, R], F32, name="ekv")
                nc.vector.tensor_tensor(ekv[:], eks[i][:], pvs[i][:],
                                        op=Op.mult)
                nc.vector.tensor_reduce(
                    nd_all[:, hb, B:2 * B],
                    ekv.rearrange("p (b t) -> p b t", t=TC),
                    axis=mybir.AxisListType.X, op=Op.add)

        # ---- AllReduce of partial num/den, one per hb-half.  The entry
        # barrier ends when the LAST core rings its first doorbell, so the
        # early half-0 doorbell moves the whole chain earlier. ----
        HH = HB // 2
        nd_sum = []

        def reduce_half(h):
            red_in = dram.tile([P, HH * 8], F32, name=f"red_in{h}")
            red_out = dram.tile([P, HH * 8], F32, name=f"red_out{h}",
                                addr_space="Shared")
            nc.sync.dma_start(red_in[:], nd_all[:, h * HH:(h + 1) * HH, :])
            nc.gpsimd.collective_compute(
                "AllReduce", Op.add,
                replica_groups=[list(range(N_CORES))],
                ins=[red_in.opt()], outs=[red_out.opt()],
            )
            ns = acts.tile([P, HH, 8], F32, name=f"nd_sum{h}")
            nc.sync.dma_start(ns[:], red_out[:])
            nd_sum.append(ns)

        for p in range(4):
            eks = kpair(p)
            vpair(p, eks)
            if p == 1:
                reduce_half(0)
        reduce_half(1)

        # ---- Q projection (fp8 DoubleRow: 2 k-chunks per matmul, halves
        # the PE instruction count) + sigmoid with the fp8 descale folded ----
        sq = []
        for hb in range(HB):
            hs = slice(hb * P, (hb + 1) * P)
            pq = psk.tile([P, R], F32, name="pq", tag="k")
            for mc2 in range(0, MC, 2):
                for rb in range(RB):
                    nc.tensor.matmul(
                        pq[:, rb * NB:(rb + 1) * NB],
                        wq[:, mc2:mc2 + 2, hs],
                        qt[:, mc2:mc2 + 2, rb * NB:(rb + 1) * NB],
                        start=(mc2 == 0), stop=(mc2 == MC - 2),
                        perf_mode=DR,
                    )
            s = sqp.tile([P, R], F16, name=f"sq{hb}")
            nc.scalar.activation(s[:], pq[:], Sigmoid,
                                 scale=1.0 / (QSCALE * WQSCALE))
            sq.append(s)

        # ---- r = num/den per half (emitted lazily: half 0 before O1,
        # half 1 before O3) ----
        r_bf = [None, None]

        def rhalf(h):
            rden = acts.tile([P, HH, B], F32, name=f"rden{h}")
            nc.vector.reciprocal(rden[:], nd_sum[h][:, :, 0:B])
            rb_ = acts.tile([P, HH, B], F16, name=f"r_bf{h}")
            nc.vector.tensor_tensor(rb_[:], nd_sum[h][:, :, B:2 * B], rden[:],
                                    op=Op.mult)
            r_bf[h] = rb_

        def fold(hc):
            # yt[hc] = sigmoid(Q)[hc] * r[hc]  (in place on sq)
            s3 = sq[hc].rearrange("p (b t) -> p b t", t=TC)
            nc.vector.tensor_tensor(
                s3, s3,
                r_bf[hc // HH][:, hc % HH, :, None].to_broadcast((P, B, TC)),
                op=Op.mult)

        def omm(po, hc, rblk, start, stop):
            rs = slice(rblk * P, (rblk + 1) * P)
            for mb in range(MB):
                nc.tensor.matmul(
                    po[:, mb * NB:(mb + 1) * NB],
                    sq[hc][:, rs],
                    wo[:, hc, mb * NB:(mb + 1) * NB],
                    start=start, stop=stop,
                )

        # ---- O1: rblk 0-3 x hc 0-3, spill to SBUF to free PSUM ----
        rhalf(0)
        po1 = [psk.tile([P, DM], F32, name="po1a", tag="k"),
               psk.tile([P, DM], F32, name="po1b", tag="k"),
               psv.tile([P, DM], F32, name="po1c", tag="v"),
               psv.tile([P, DM], F32, name="po1d", tag="v")]
        for hc in range(HH):
            fold(hc)
            for rblk in range(4):
                omm(po1[rblk], hc, rblk, start=(hc == 0), stop=(hc == HH - 1))
        ob = []
        for rblk in range(4):
            o = obp.tile([P, DM], F32, name=f"ob{rblk}")
            if rblk % 2 == 0:
                nc.vector.tensor_copy(o[:], po1[rblk][:])
            else:
                nc.scalar.copy(o[:], po1[rblk][:])
            ob.append(o)

        # ---- O2: rblk 4-7 x hc 0-3, held open in PSUM across the AR1 wait
        po2 = [psk.tile([P, DM], F32, name="po2a", tag="k"),
               psk.tile([P, DM], F32, name="po2b", tag="k"),
               psv.tile([P, DM], F32, name="po2c", tag="v"),
               psv.tile([P, DM], F32, name="po2d", tag="v")]
        for hc in range(HH):
            for rblk in range(4, 8):
                omm(po2[rblk - 4], hc, rblk, start=(hc == 0), stop=False)

        # ---- O3: rblk 4-7 x hc 4-7 finish + store ----
        rhalf(1)
        for hc in range(HH, HB):
            fold(hc)
            for rblk in range(4, 8):
                omm(po2[rblk - 4], hc, rblk, start=False, stop=(hc == HB - 1))
        for rblk in range(4, 8):
            rs = slice(rblk * P, (rblk + 1) * P)
            ot = work.tile([P, DM], F32, name="ot")
            if rblk % 2 == 0:
                nc.vector.tensor_copy(ot[:], po2[rblk - 4][:])
            else:
                nc.scalar.copy(ot[:], po2[rblk - 4][:])
            nc.sync.dma_start(out[rs, :], ot[:])

        # ---- O4: rblk 0-3 x hc 4-7 + add of the O1 spill + store ----
        po4 = [psk.tile([P, DM], F32, name="po4a", tag="k"),
               psk.tile([P, DM], F32, name="po4b", tag="k"),
               psv.tile([P, DM], F32, name="po4c", tag="v"),
               psv.tile([P, DM], F32, name="po4d", tag="v")]
        for hc in range(HH, HB):
            for rblk in range(4):
                omm(po4[rblk], hc, rblk, start=(hc == HH), stop=(hc == HB - 1))
        for rblk in range(4):
            rs = slice(rblk * P, (rblk + 1) * P)
            ot = work.tile([P, DM], F32, name="ot")
            if rblk == 3:
                # split the last block to shorten the DMA tail
                nc.vector.tensor_tensor(ot[:, 0:NB], ob[rblk][:, 0:NB],
                                        po4[rblk][:, 0:NB], op=Op.add)
                nc.sync.dma_start(out[rs, 0:NB], ot[:, 0:NB])
                nc.vector.tensor_tensor(ot[:, NB:DM], ob[rblk][:, NB:DM],
                                        po4[rblk][:, NB:DM], op=Op.add)
                nc.sync.dma_start(out[rs, NB:DM], ot[:, NB:DM])
            else:
                nc.vector.tensor_tensor(ot[:], ob[rblk][:], po4[rblk][:],
                                        op=Op.add)
                nc.sync.dma_start(out[rs, :], ot[:])


def _dedup_ldweights(nc):
    """Drop InstLdweights whose weight AP is identical to the PE's already-
    loaded weights (the 2nd matmul of each rb-pair reloads the same tile).
    Each reload costs ~50ns of weight-plane fill serialized into the next
    matmul (263ns vs 216ns per MM measured), so halving LDW count saves
    ~12-25us across 512 matmuls.  Waits/updates of a dropped LDW are merged
    into the following PE instruction (fires later -> still safe)."""
    PE = mybir.EngineType.PE
    for f in nc.m.functions:
        for blk in f.blocks:
            insts = list(blk.instructions)
            keep = []
            last_sig = None
            pending_si = None
            for inst in insts:
                if inst.engine == PE:
                    if isinstance(inst, mybir.InstLdweights):
                        sig = (str(inst.ins[0]), str(inst.tile_position),
                               str(inst.perf_mode), str(inst.is_transpose))
                        if sig == last_sig:
                            si = inst.sync_info
                            if si is not None and (si.on_wait or si.on_update):
                                if pending_si is None:
                                    pending_si = ([], [])
                                pending_si[0].extend(si.on_wait)
                                pending_si[1].extend(si.on_update)
                            continue  # drop redundant reload
                        last_sig = sig
                    elif isinstance(inst, mybir.InstMatmult):
                        if pending_si is not None:
                            si = inst.sync_info
                            if si is None:
                                si = mybir.SyncInfo(on_wait=[], on_update=[])
                            inst.sync_info = mybir.SyncInfo(
                                on_wait=list(si.on_wait) + pending_si[0],
                                on_update=list(si.on_update) + pending_si[1],
                            )
                            pending_si = None
                    elif isinstance(inst, (mybir.InstEventSemaphore,
                                           mybir.InstNoOp, mybir.InstDrain)):
                        pass  # sequencer-only ops don't touch the PE array
                    else:
                        last_sig = None  # unknown PE op: be conservative
                keep.append(inst)
            assert pending_si is None
            if len(keep) != len(insts):
                blk.instructions[:] = keep


def _build():
    global _GRAPH
    if _GRAPH is None:
        nc = bacc.Bacc("TRN2", target_bir_lowering=False, debug=False,
                       num_devices=N_CORES)
        with tile.TileContext(nc) as tc:
            _body(nc, tc)
        _dedup_ldweights(nc)
        nc.compile()
        _GRAPH = nc
    return _GRAPH


def _shard_inputs(inputs):
    q = np.asarray(inputs["q"], np.float32)
    k = np.asarray(inputs["k"], np.float32)
    v = np.asarray(inputs["v"], np.float32)
    wqT = np.ascontiguousarray(
        np.asarray(inputs["Wq"], np.float32).T * WQSCALE).astype(NPF8)
    wkT = np.ascontiguousarray(np.asarray(inputs["Wk"], np.float32).T).astype(NPF16)
    wvT = np.ascontiguousarray(np.asarray(inputs["Wv"], np.float32).T).astype(NPF16)
    woT = np.ascontiguousarray(np.asarray(inputs["Wo"], np.float32).T).astype(NPF16)

    def tslice(x, c, dt=NPF16, scale=None):
        # (B, TC, DM) -> (DM, B, TC) -> (DM, R) with r = b*256 + t
        s = x[:, c * TC:(c + 1) * TC, :].transpose(2, 0, 1)
        s = np.ascontiguousarray(s).reshape(DM, R)
        if scale is not None:
            s = s * scale
        return s.astype(dt)

    in_maps = []
    for c in range(N_CORES):
        in_maps.append({
            "qT": tslice(q, c, NPF8, QSCALE),
            "kT": tslice(k, c),
            "vT": tslice(v, c),
            "wqT": wqT, "wkT": wkT, "wvT": wvT, "woT": woT,
        })
    return in_maps


def _unshard(outs):
    full = np.empty((B, T, DM), np.float32)
    for c in range(N_CORES):
        # out_c[r, m] with r = b*256 + t  ->  (b, t, m)
        full[:, c * TC:(c + 1) * TC, :] = outs[c].reshape(B, TC, DM)
    return full


def run(inputs, trace=False, trace_cores=None, **kw):
    nc = _build()
    in_maps = _shard_inputs(inputs)
    res = run_bass_kernel_spmd(
        nc, in_maps, list(range(N_CORES)),
        trace=trace, trace_cores=trace_cores, **kw)
    return _unshard([m["out"] for m in res.results]), res


def kernel(**inputs):
    out, _ = run(inputs)
    return out


# revision 7
# speedup vs baseline: 1.0442x; 1.0442x over previous
"""AFT-Full attention kernel for 8 TRN2 NeuronCores.

Math: the reference's exp_pos_bias = exp(pos_bias - max(pos_bias, axis=0)) is
identically 1.0 (the max is over a singleton dim), so the two (b,Ti,Tj,Dh)
einsums collapse to per-(b,h) sums over j:
    num[b,h] = sum_j exp(K-max_b K)[b,j,h] * V[b,j,h]
    den[b,h] = sum_j exp(K-max_b K)[b,j,h]
    out = (sigmoid(Q) * num/den) @ Wo.T

Sharding: sequence-parallel over T (256 positions per core, all 4 batches),
weights replicated.  Per-core row index r = b*256 + t_local (b-major) so the
per-(b,h) sums over t are contiguous innermost reduces.  One AllReduce per
half of d_hidden combines the per-core partial num/den.

Schedule (v2): K/V projections run as hb-PAIRS with the contraction (mc)
loop OUTER so the first matmuls start as soon as the first 128-row chunk of
kt/wk lands (instead of waiting for the full 4MB).  num/den for hb 0-3
complete after pair 1 -> AllReduce half 0 doorbell rings ~35us earlier than
the v1 schedule; the collectives entry barrier ends when the LAST core rings
its first doorbell, so earlier doorbells move the whole collective chain
earlier.  The output projection runs in 4 phases so the PE never idles
between the two AllReduce completions:
  O1: rblk 0-3 x hc 0-3 (needs r half 0), spilled to SBUF (frees PSUM)
  O2: rblk 4-7 x hc 0-3, held open in PSUM across the AR1 wait
  O3: rblk 4-7 x hc 4-7 finish + store          (needs r half 1)
  O4: rblk 0-3 x hc 4-7 + DVE add of O1 spill + store
"""

import numpy as np
import ml_dtypes

import concourse.bass as bass
import concourse.mybir as mybir
import concourse.tile as tile
from concourse import bacc
from concourse.bass_utils import run_bass_kernel_spmd

B, T, DM, DH = 4, 2048, 1024, 1024
N_CORES = 8
TC = T // N_CORES          # 256 sequence positions per core
R = B * TC                 # 1024 rows per core, r = b*256 + t
P = 128
MC = DM // P               # 8 contraction chunks (d_model)
HB = DH // P               # 8 hidden blocks
NB = 512                   # matmul moving free dim
RB = R // NB               # 2 row blocks
MB = DM // NB              # 2 output-model blocks

F16 = mybir.dt.bfloat16
F8 = mybir.dt.float8e4
F32 = mybir.dt.float32
NPF16 = ml_dtypes.bfloat16
NPF8 = ml_dtypes.float8_e4m3   # TRN FP8_EXP4: max +-240, matches in range
# fp8 pre-scales for the Q path (descaled inside the sigmoid activation).
# q ~ N(0,1) * 16 -> +-88 max; Wq ~ N(0,0.02^2) * 512 -> +-56 max: no clip,
# negligible subnormals.  Sigmoid damps the fp8 quantization 4x; simulated
# end-to-end rel err 1.07e-2 vs the 2e-2 gate.
QSCALE = 16.0
WQSCALE = 512.0

_GRAPH = None


def _body(nc, tc):
    qT = nc.dram_tensor("qT", [DM, R], F8, kind="ExternalInput").ap()
    kT = nc.dram_tensor("kT", [DM, R], F16, kind="ExternalInput").ap()
    vT = nc.dram_tensor("vT", [DM, R], F16, kind="ExternalInput").ap()
    wqT = nc.dram_tensor("wqT", [DM, DH], F8, kind="ExternalInput").ap()
    wkT = nc.dram_tensor("wkT", [DM, DH], F16, kind="ExternalInput").ap()
    wvT = nc.dram_tensor("wvT", [DM, DH], F16, kind="ExternalInput").ap()
    woT = nc.dram_tensor("woT", [DH, DM], F16, kind="ExternalInput").ap()
    out = nc.dram_tensor("out", [R, DM], F32, kind="ExternalOutput").ap()

    Exp = mybir.ActivationFunctionType.Exp
    Sigmoid = mybir.ActivationFunctionType.Sigmoid
    Op = mybir.AluOpType
    DR = mybir.MatmulPerfMode.DoubleRow

    from contextlib import ExitStack
    with ExitStack() as ctx:
        acts = ctx.enter_context(tc.tile_pool(name="acts", bufs=1))
        work = ctx.enter_context(tc.tile_pool(name="work", bufs=2))
        sqp = ctx.enter_context(tc.tile_pool(name="sqp", bufs=1))
        obp = ctx.enter_context(tc.tile_pool(name="obp", bufs=1))
        # 2 + 2 [128,1024]f32 buffers = all 8 PSUM banks.  K pairs use the
        # two "k" slots, V pairs the two "v" slots; the O phases use all 4.
        psk = ctx.enter_context(tc.tile_pool(name="psk", bufs=2, space="PSUM"))
        psv = ctx.enter_context(tc.tile_pool(name="psv", bufs=2, space="PSUM"))
        dram = ctx.enter_context(tc.tile_pool(name="dram", bufs=1, space="DRAM"))

        # Dummy 4-byte AllReduce, first thing on the GpSimd queue: the
        # collectives entry barrier ends ~10us after the LAST core rings its
        # FIRST doorbell, so ringing at ~10us (instead of when num/den are
        # ready at ~40us) pulls the whole collective chain forward.  Its
        # result is unused; the real AllReduces queue behind it on the same
        # stream, by which time it has long completed.
        dum_in = dram.tile([1, 1], F32, name="dum_in")
        dum_out = dram.tile([1, 1], F32, name="dum_out", addr_space="Shared")
        nc.gpsimd.collective_compute(
            "AllReduce", mybir.AluOpType.add,
            replica_groups=[list(range(N_CORES))],
            ins=[dum_in.opt()], outs=[dum_out.opt()],
        )

        # Warmup matmuls on zeros un-throttle the PE clock (HAM) while the
        # first input chunks stream in; a copy of their result is stored to
        # a DRAM scratch tile (at kernel end, so the dump DMA does not sit
        # at the head of the sync ring blocking the input stream).
        warm = acts.tile([P, 640], F16, name="warm")
        nc.gpsimd.memset(warm[:], 0.0)
        pwu = psk.tile([P, NB], F32, name="pwu", tag="k")
        pwu2 = psv.tile([P, NB], F32, name="pwu2", tag="v")
        for i in range(8):
            t = pwu if i % 2 == 0 else pwu2
            nc.tensor.matmul(t[:], warm[:, 0:P], warm[:, P:P + NB],
                             start=True, stop=True)
        wdump = acts.tile([1, 2], F32, name="wdump")
        nc.vector.tensor_copy(wdump[:, 0:1], pwu[0:1, 0:1])
        nc.vector.tensor_copy(wdump[:, 1:2], pwu2[0:1, 0:1])

        def declare(name, free, dt=F16):
            return acts.tile([P, MC, free], dt, name=name)

        def load_chunk(t, ap_dram, mc):
            src = ap_dram.rearrange("(c p) f -> p c f", p=P)
            nc.sync.dma_start(t[:, mc, :], src[:, mc, :])

        # K/V operands stream in per-mc on the sync HWDGE ring (~270 GB/s);
        # the mc-outer matmul emission below consumes each chunk as it
        # lands.  (scalar.dma_start is a slow serialized DMA_DIRECT2D — do
        # not route bulk loads through it.)
        kt = declare("kt", R)
        wk = declare("wk", DH)
        for mc in range(MC):
            load_chunk(wk, wkT, mc)
            load_chunk(kt, kT, mc)
        vt = declare("vt", R)
        wv = declare("wv", DH)
        for mc in range(MC):
            load_chunk(wv, wvT, mc)
            load_chunk(vt, vT, mc)
        qt = declare("qt", R, F8)
        nc.sync.dma_start(qt[:], qT.rearrange("(c p) f -> p c f", p=P))
        wq = declare("wq", DH, F8)
        nc.sync.dma_start(wq[:], wqT.rearrange("(c p) f -> p c f", p=P))
        wo = declare("wo", DM)
        nc.sync.dma_start(wo[:], woT.rearrange("(c p) f -> p c f", p=P))

        nd_all = acts.tile([P, HB, 8], F32, name="nd_all")

        def kpost(hb, pk):
            # max over batch: copy + 3 maxes (DVE may read PSUM only once/op)
            mk = work.tile([P, TC], F32, name="mk")
            nc.vector.tensor_copy(mk[:], pk[:, 0 * TC:1 * TC])
            for b in range(1, B):
                nc.vector.tensor_tensor(
                    mk[:], mk[:], pk[:, b * TC:(b + 1) * TC], op=Op.max)
            ek = work.tile([P, R], F32, name="ek")
            ek3 = ek.rearrange("p (b t) -> p b t", t=TC)
            nc.vector.tensor_tensor(
                ek3, pk.rearrange("p (b t) -> p b t", t=TC),
                mk[:, None, :].to_broadcast((P, B, TC)), op=Op.subtract)
            # exp on ACT with fused per-b den accumulation (contiguous slices)
            for b in range(B):
                bs = slice(b * TC, (b + 1) * TC)
                nc.scalar.activation(
                    ek[:, bs], ek[:, bs], Exp,
                    accum_out=nd_all[:, hb, b:b + 1])
            return ek

        # ---- K pair: projections for hb-pair (2p, 2p+1) with mc OUTER so
        # the matmuls chase the kt/wk DMA stream chunk by chunk. ----
        def kpair(p):
            pks = [psk.tile([P, R], F32, name="pk", tag="k") for _ in range(2)]
            for mc in range(MC):
                for i in range(2):
                    hs = slice((2 * p + i) * P, (2 * p + i + 1) * P)
                    for rb in range(RB):
                        nc.tensor.matmul(
                            pks[i][:, rb * NB:(rb + 1) * NB],
                            wk[:, mc, hs],
                            kt[:, mc, rb * NB:(rb + 1) * NB],
                            start=(mc == 0), stop=(mc == MC - 1),
                        )
            return [kpost(2 * p + i, pks[i]) for i in range(2)]

        def vpair(p, eks):
            pvs = [psv.tile([P, R], F32, name="pv", tag="v") for _ in range(2)]
            for mc in range(MC):
                for i in range(2):
                    hs = slice((2 * p + i) * P, (2 * p + i + 1) * P)
                    for rb in range(RB):
                        nc.tensor.matmul(
                            pvs[i][:, rb * NB:(rb + 1) * NB],
                            wv[:, mc, hs],
                            vt[:, mc, rb * NB:(rb + 1) * NB],
                            start=(mc == 0), stop=(mc == MC - 1),
                        )
            for i in range(2):
                hb = 2 * p + i
                ekv = work.tile([P, R], F32, name="ekv")
                nc.vector.tensor_tensor(ekv[:], eks[i][:], pvs[i][:],
                                        op=Op.mult)
                nc.vector.tensor_reduce(
                    nd_all[:, hb, B:2 * B],
                    ekv.rearrange("p (b t) -> p b t", t=TC),
                    axis=mybir.AxisListType.X, op=Op.add)

        # ---- AllReduce of partial num/den, one per hb-half.  The entry
        # barrier ends when the LAST core rings its first doorbell, so the
        # early half-0 doorbell moves the whole chain earlier. ----
        HH = HB // 2
        nd_sum = []

        def reduce_half(h):
            red_in = dram.tile([P, HH * 8], F32, name=f"red_in{h}")
            red_out = dram.tile([P, HH * 8], F32, name=f"red_out{h}",
                                addr_space="Shared")
            # scalar DMA_DIRECT2D: tiny 16KB transfer, skips the sync ring
            # which is still draining the 12.6MB input stream at this point
            nc.scalar.dma_start(red_in[:], nd_all[:, h * HH:(h + 1) * HH, :])
            nc.gpsimd.collective_compute(
                "AllReduce", Op.add,
                replica_groups=[list(range(N_CORES))],
                ins=[red_in.opt()], outs=[red_out.opt()],
            )
            ns = acts.tile([P, HH, 8], F32, name=f"nd_sum{h}")
            nc.sync.dma_start(ns[:], red_out[:])
            nd_sum.append(ns)

        for p in range(4):
            eks = kpair(p)
            vpair(p, eks)
            if p == 1:
                reduce_half(0)
        reduce_half(1)

        # ---- Q projection (fp8 DoubleRow: 2 k-chunks per matmul, halves
        # the PE instruction count) + sigmoid with the fp8 descale folded ----
        sq = []
        for hb in range(HB):
            hs = slice(hb * P, (hb + 1) * P)
            pq = psk.tile([P, R], F32, name="pq", tag="k")
            for mc2 in range(0, MC, 2):
                for rb in range(RB):
                    nc.tensor.matmul(
                        pq[:, rb * NB:(rb + 1) * NB],
                        wq[:, mc2:mc2 + 2, hs],
                        qt[:, mc2:mc2 + 2, rb * NB:(rb + 1) * NB],
                        start=(mc2 == 0), stop=(mc2 == MC - 2),
                        perf_mode=DR,
                    )
            s = sqp.tile([P, R], F16, name=f"sq{hb}")
            nc.scalar.activation(s[:], pq[:], Sigmoid,
                                 scale=1.0 / (QSCALE * WQSCALE))
            sq.append(s)

        # ---- r = num/den per half (emitted lazily: half 0 before O1,
        # half 1 before O3) ----
        r_bf = [None, None]

        def rhalf(h):
            rden = acts.tile([P, HH, B], F32, name=f"rden{h}")
            nc.vector.reciprocal(rden[:], nd_sum[h][:, :, 0:B])
            rb_ = acts.tile([P, HH, B], F16, name=f"r_bf{h}")
            nc.vector.tensor_tensor(rb_[:], nd_sum[h][:, :, B:2 * B], rden[:],
                                    op=Op.mult)
            r_bf[h] = rb_

        def fold(hc):
            # yt[hc] = sigmoid(Q)[hc] * r[hc]  (in place on sq)
            s3 = sq[hc].rearrange("p (b t) -> p b t", t=TC)
            nc.vector.tensor_tensor(
                s3, s3,
                r_bf[hc // HH][:, hc % HH, :, None].to_broadcast((P, B, TC)),
                op=Op.mult)

        def omm(po, hc, rblk, start, stop):
            rs = slice(rblk * P, (rblk + 1) * P)
            for mb in range(MB):
                nc.tensor.matmul(
                    po[:, mb * NB:(mb + 1) * NB],
                    sq[hc][:, rs],
                    wo[:, hc, mb * NB:(mb + 1) * NB],
                    start=start, stop=stop,
                )

        # ---- O1: rblk 0-3 x hc 0-3, spill to SBUF to free PSUM ----
        rhalf(0)
        po1 = [psk.tile([P, DM], F32, name="po1a", tag="k"),
               psk.tile([P, DM], F32, name="po1b", tag="k"),
               psv.tile([P, DM], F32, name="po1c", tag="v"),
               psv.tile([P, DM], F32, name="po1d", tag="v")]
        for hc in range(HH):
            fold(hc)
            for rblk in range(4):
                omm(po1[rblk], hc, rblk, start=(hc == 0), stop=(hc == HH - 1))
        ob = []
        for rblk in range(4):
            o = obp.tile([P, DM], F32, name=f"ob{rblk}")
            if rblk % 2 == 0:
                nc.vector.tensor_copy(o[:], po1[rblk][:])
            else:
                nc.scalar.copy(o[:], po1[rblk][:])
            ob.append(o)

        # ---- O2: rblk 4-7 x hc 0-3, held open in PSUM across the AR1 wait
        po2 = [psk.tile([P, DM], F32, name="po2a", tag="k"),
               psk.tile([P, DM], F32, name="po2b", tag="k"),
               psv.tile([P, DM], F32, name="po2c", tag="v"),
               psv.tile([P, DM], F32, name="po2d", tag="v")]
        for hc in range(HH):
            for rblk in range(4, 8):
                omm(po2[rblk - 4], hc, rblk, start=(hc == 0), stop=False)

        # ---- O3: rblk 4-7 x hc 4-7 finish + store ----
        rhalf(1)
        for hc in range(HH, HB):
            fold(hc)
            for rblk in range(4, 8):
                omm(po2[rblk - 4], hc, rblk, start=False, stop=(hc == HB - 1))
        for rblk in range(4, 8):
            rs = slice(rblk * P, (rblk + 1) * P)
            ot = work.tile([P, DM], F32, name="ot")
            if rblk % 2 == 0:
                nc.vector.tensor_copy(ot[:], po2[rblk - 4][:])
            else:
                nc.scalar.copy(ot[:], po2[rblk - 4][:])
            nc.sync.dma_start(out[rs, :], ot[:])

        # ---- O4: rblk 0-3 x hc 4-7 + add of the O1 spill + store ----
        po4 = [psk.tile([P, DM], F32, name="po4a", tag="k"),
               psk.tile([P, DM], F32, name="po4b", tag="k"),
               psv.tile([P, DM], F32, name="po4c", tag="v"),
               psv.tile([P, DM], F32, name="po4d", tag="v")]
        for hc in range(HH, HB):
            for rblk in range(4):
                omm(po4[rblk], hc, rblk, start=(hc == HH), stop=(hc == HB - 1))
        for rblk in range(4):
            rs = slice(rblk * P, (rblk + 1) * P)
            ot = work.tile([P, DM], F32, name="ot")
            if rblk == 3:
                # split the last block to shorten the DMA tail
                nc.vector.tensor_tensor(ot[:, 0:NB], ob[rblk][:, 0:NB],
                                        po4[rblk][:, 0:NB], op=Op.add)
                nc.sync.dma_start(out[rs, 0:NB], ot[:, 0:NB])
                nc.vector.tensor_tensor(ot[:, NB:DM], ob[rblk][:, NB:DM],
                                        po4[rblk][:, NB:DM], op=Op.add)
                nc.sync.dma_start(out[rs, NB:DM], ot[:, NB:DM])
            else:
                nc.vector.tensor_tensor(ot[:], ob[rblk][:], po4[rblk][:],
                                        op=Op.add)
                nc.sync.dma_start(out[rs, :], ot[:])

        # warmup-dump store, deferred to keep the sync ring head clear
        wscratch = dram.tile([1, 2], F32, name="wscratch")
        nc.scalar.dma_start(wscratch[:], wdump[:])


def _dedup_ldweights(nc):
    """Drop InstLdweights whose weight AP is identical to the PE's already-
    loaded weights (the 2nd matmul of each rb-pair reloads the same tile).
    Each reload costs ~50ns of weight-plane fill serialized into the next
    matmul (263ns vs 216ns per MM measured), so halving LDW count saves
    ~12-25us across 512 matmuls.  Waits/updates of a dropped LDW are merged
    into the following PE instruction (fires later -> still safe)."""
    PE = mybir.EngineType.PE
    for f in nc.m.functions:
        for blk in f.blocks:
            insts = list(blk.instructions)
            keep = []
            last_sig = None
            pending_si = None
            for inst in insts:
                if inst.engine == PE:
                    if isinstance(inst, mybir.InstLdweights):
                        sig = (str(inst.ins[0]), str(inst.tile_position),
                               str(inst.perf_mode), str(inst.is_transpose))
                        if sig == last_sig:
                            si = inst.sync_info
                            if si is not None and (si.on_wait or si.on_update):
                                if pending_si is None:
                                    pending_si = ([], [])
                                pending_si[0].extend(si.on_wait)
                                pending_si[1].extend(si.on_update)
                            continue  # drop redundant reload
                        last_sig = sig
                    elif isinstance(inst, mybir.InstMatmult):
                        if pending_si is not None:
                            si = inst.sync_info
                            if si is None:
                                si = mybir.SyncInfo(on_wait=[], on_update=[])
                            inst.sync_info = mybir.SyncInfo(
                                on_wait=list(si.on_wait) + pending_si[0],
                                on_update=list(si.on_update) + pending_si[1],
                            )
                            pending_si = None
                    elif isinstance(inst, (mybir.InstEventSemaphore,
                                           mybir.InstNoOp, mybir.InstDrain)):
                        pass  # sequencer-only ops don't touch the PE array
                    else:
                        last_sig = None  # unknown PE op: be conservative
                keep.append(inst)
            assert pending_si is None
            if len(keep) != len(insts):
                blk.instructions[:] = keep


def _build():
    global _GRAPH
    if _GRAPH is None:
        nc = bacc.Bacc("TRN2", target_bir_lowering=False, debug=False,
                       num_devices=N_CORES)
        with tile.TileContext(nc) as tc:
            _body(nc, tc)
        _dedup_ldweights(nc)
        nc.compile()
        _GRAPH = nc
    return _GRAPH


def _shard_inputs(inputs):
    q = np.asarray(inputs["q"], np.float32)
    k = np.asarray(inputs["k"], np.float32)
    v = np.asarray(inputs["v"], np.float32)
    wqT = np.ascontiguousarray(
        np.asarray(inputs["Wq"], np.float32).T * WQSCALE).astype(NPF8)
    wkT = np.ascontiguousarray(np.asarray(inputs["Wk"], np.float32).T).astype(NPF16)
    wvT = np.ascontiguousarray(np.asarray(inputs["Wv"], np.float32).T).astype(NPF16)
    woT = np.ascontiguousarray(np.asarray(inputs["Wo"], np.float32).T).astype(NPF16)

    def tslice(x, c, dt=NPF16, scale=None):
        # (B, TC, DM) -> (DM, B, TC) -> (DM, R) with r = b*256 + t
        s = x[:, c * TC:(c + 1) * TC, :].transpose(2, 0, 1)
        s = np.ascontiguousarray(s).reshape(DM, R)
        if scale is not None:
            s = s * scale
        return s.astype(dt)

    in_maps = []
    for c in range(N_CORES):
        in_maps.append({
            "qT": tslice(q, c, NPF8, QSCALE),
            "kT": tslice(k, c),
            "vT": tslice(v, c),
            "wqT": wqT, "wkT": wkT, "wvT": wvT, "woT": woT,
        })
    return in_maps


def _unshard(outs):
    full = np.empty((B, T, DM), np.float32)
    for c in range(N_CORES):
        # out_c[r, m] with r = b*256 + t  ->  (b, t, m)
        full[:, c * TC:(c + 1) * TC, :] = outs[c].reshape(B, TC, DM)
    return full


def run(inputs, trace=False, trace_cores=None, **kw):
    nc = _build()
    in_maps = _shard_inputs(inputs)
    res = run_bass_kernel_spmd(
        nc, in_maps, list(range(N_CORES)),
        trace=trace, trace_cores=trace_cores, **kw)
    return _unshard([m["out"] for m in res.results]), res


def kernel(**inputs):
    out, _ = run(inputs)
    return out
